# revision 15
# baseline (speedup 1.0000x reference)
"""GQA prefill attention (B=1, T=2048, DIM=4096, 32 q-heads / 8 kv-heads),
tensor-parallel over 8 NeuronCores.

Sharding: core c owns kv head c and its 4 query heads: wq rows
[512c, 512c+512), wk/wv rows [128c, 128c+128), wo cols [512c, 512c+512).
Each core computes a partial y = attn_c @ wo_c.T in [T, DIM]; the host sums
the 8 partials (the "all-reduce after wo").

All matmul operands are bf16 (PSUM accumulation in f32); the PE streams
1 row/cycle for bf16, so precision is traded only for DMA bandwidth.
Everything on chip is kept transposed ([feature, t] layouts) so the PE
contraction dim (partitions) always lines up; the only transpose is V
(16 PE-transposes of 128x128).

Per-core pipeline:
  phase 1: Q^T/K^T/V^T projections from x^T (host-pretransposed bf16)
  phase 2+3 fused, per t-chunk j = 3..0:
    attention for all 4 heads of chunk j (S^T tiles -> exp -> rowsum via
    ones-matmul -> O^T accumulation), with NORMALIZATION DEFERRED: the
    per-chunk row-sums are batch-reciprocated off the critical path
    (DRAM-bounce transpose -> [128,16] reciprocal -> bounce back ->
    partition-broadcast reads -> in-place scale of ao), overlapping the
    next chunk's attention; out-projection + y write of chunk j runs
    after attention of chunk j-1, so y DMA overlaps attention compute.
"""

import sys

sys.path.insert(0, "/opt/trn_rl_repo")

import numpy as np

import concourse.bass as bass
import concourse.tile as tile
from concourse import bacc, mybir
from concourse.bass_utils import run_bass_kernel_spmd
from concourse.masks import make_identity

T = 2048
DIM = 4096
HD = 128
NCORE = 8
NH = 4  # q heads per core
TCH = 512
NTCH = T // TCH  # 4 t-chunks
NST = T // 128  # 16 s-tiles
NDT = DIM // 128  # 32 d-tiles
F32 = mybir.dt.float32
BF16 = mybir.dt.bfloat16
SCALE = 1.0 / float(np.sqrt(HD))
NEG = -1e30

# test.py can flip these before calling kernel() to get profiling info
TRACE = False
LAST = {}

_CACHE = {}


def _build():
    nc = bacc.Bacc("TRN2", target_bir_lowering=False, debug=False, num_devices=NCORE)
    xT = nc.dram_tensor("xT", [DIM, T], BF16, kind="ExternalInput").ap()
    wqT = nc.dram_tensor("wqT", [DIM, NH * HD], BF16, kind="ExternalInput").ap()
    wkT = nc.dram_tensor("wkT", [DIM, HD], BF16, kind="ExternalInput").ap()
    wvT = nc.dram_tensor("wvT", [DIM, HD], BF16, kind="ExternalInput").ap()
    woT = nc.dram_tensor("woT", [NH * HD, DIM], BF16, kind="ExternalInput").ap()
    ones_in = nc.dram_tensor("ones", [128, 1], BF16, kind="ExternalInput").ap()
    y = nc.dram_tensor("y", [T, DIM], BF16, kind="ExternalOutput").ap()

    with tile.TileContext(nc) as tc:
        with tc.tile_pool(name="persist", bufs=1) as persist:
            qt_sb = [persist.tile([128, T], BF16, tag=f"qt{h}", name=f"qt{h}") for h in range(NH)]
            kt_sb = persist.tile([128, T], BF16, tag="kt")
            vt_sb = persist.tile([128, T], BF16, tag="vt")
            ao_sb = [persist.tile([128, T], BF16, tag=f"ao{h}", name=f"ao{h}") for h in range(NH)]
            ones_sb = persist.tile([128, 1], BF16, tag="ones")
            ident = persist.tile([128, 128], BF16, tag="ident")
            v_sb = persist.tile([128, NST, HD], BF16, tag="v")
            nc.sync.dma_start(out=ones_sb, in_=ones_in)
            make_identity(nc, ident)

            # ---------------- phase 1: Q/K/V projections ----------------
            with (
                tc.tile_pool(name="w1", bufs=1) as w1,
                tc.tile_pool(name="xs", bufs=6) as xs,
                tc.tile_pool(name="psp", bufs=1, space="PSUM") as psp,
                tc.tile_pool(name="ptr1", bufs=1, space="PSUM") as ptr1,
            ):
                wq_sb = w1.tile([128, NDT, NH * HD], BF16, tag="wq")
                wk_sb = w1.tile([128, NDT, HD], BF16, tag="wk")
                wv_sb = w1.tile([128, NDT, HD], BF16, tag="wv")
                wqr = wqT.rearrange("(db p) f -> p db f", p=128)
                wkr = wkT.rearrange("(db p) f -> p db f", p=128)
                wvr = wvT.rearrange("(db p) f -> p db f", p=128)
                # d=0 weights + first x tile first (PE starts on the q
                # matmuls almost immediately), then the rest of the weights
                # in 512KB groups; the sync queue streams them well ahead
                # of the PE's ~5us-per-group consumption
                g0 = slice(0, 1)
                nc.sync.dma_start(out=wq_sb[:, g0, :], in_=wqr[:, g0, :])
                xt0 = xs.tile([128, TCH], BF16, tag="xt")
                nc.sync.dma_start(out=xt0, in_=xT[0:128, 0:TCH])
                nc.sync.dma_start(out=wk_sb[:, g0, :], in_=wkr[:, g0, :])
                nc.sync.dma_start(out=wv_sb[:, g0, :], in_=wvr[:, g0, :])
                for g in [slice(1, 4)] + [slice(d, d + 4) for d in range(4, NDT, 4)]:
                    nc.sync.dma_start(out=wq_sb[:, g, :], in_=wqr[:, g, :])
                    nc.sync.dma_start(out=wk_sb[:, g, :], in_=wkr[:, g, :])
                    nc.sync.dma_start(out=wv_sb[:, g, :], in_=wvr[:, g, :])
                for ch in range(NTCH):
                    cs = slice(ch * TCH, (ch + 1) * TCH)
                    qps = [
                        psp.tile([128, TCH], F32, tag=f"projq{fq}", name=f"projq{fq}")
                        for fq in range(NH)
                    ]
                    kps = psp.tile([128, TCH], F32, tag="projk")
                    vps = psp.tile([128, TCH], F32, tag="projv")
                    for d in range(NDT):
                        if ch == 0 and d == 0:
                            xt = xt0
                        else:
                            xt = xs.tile([128, TCH], BF16, tag="xt")
                            nc.sync.dma_start(
                                out=xt,
                                in_=xT[d * 128 : (d + 1) * 128, cs],
                            )
                        st = d == 0
                        sp = d == NDT - 1
                        for fq in range(NH):
                            nc.tensor.matmul(
                                qps[fq][:],
                                wq_sb[:, d, fq * HD : (fq + 1) * HD],
                                xt[:],
                                start=st,
                                stop=sp,
                            )
                        nc.tensor.matmul(
                            kps[:], wk_sb[:, d, :], xt[:], start=st, stop=sp
                        )
                        nc.tensor.matmul(
                            vps[:], wv_sb[:, d, :], xt[:], start=st, stop=sp
                        )
                    for fq in range(NH):
                        nc.vector.tensor_copy(qt_sb[fq][:, cs], qps[fq][:])
                    nc.vector.tensor_copy(kt_sb[:, cs], kps[:])
                    nc.vector.tensor_copy(vt_sb[:, cs], vps[:])
                    for ii in range(4 * ch, 4 * ch + 4):
                        ptr = ptr1.tile([128, HD], BF16, tag="tr")
                        nc.tensor.transpose(
                            ptr[:], vt_sb[:, ii * 128 : (ii + 1) * 128], ident[:]
                        )
                        nc.vector.tensor_copy(v_sb[:, ii, :], ptr[:])

            # ---------------- phase 2+3: attention + out-projection ----------------
            with (
                tc.tile_pool(name="w2", bufs=1) as w2,
                tc.tile_pool(name="phatp", bufs=6) as phatp,
                tc.tile_pool(name="recipp", bufs=2) as recipp,
                tc.tile_pool(name="rbcp", bufs=3) as rbcp,
                tc.tile_pool(name="rdram", bufs=2, space="DRAM") as rdram,
                tc.tile_pool(name="ys", bufs=4) as ys,
                tc.tile_pool(name="maskedp", bufs=3) as maskedp,
                tc.tile_pool(name="pst", bufs=2, space="PSUM") as pst,
                tc.tile_pool(name="psl", bufs=2, space="PSUM") as psl,
                tc.tile_pool(name="psot", bufs=2, space="PSUM") as psot,
                tc.tile_pool(name="psy", bufs=2, space="PSUM") as psy,
            ):
                wo_sb = w2.tile([128, NH, DIM], BF16, tag="wo")
                wor = woT.rearrange("(hb p) f -> p hb f", p=128)
                maskneg = w2.tile([128, 4, TCH], F32, tag="maskneg")
                nc.vector.memset(maskneg, 0.0)
                for r in range(4):
                    # keep where (t - s - 128r) >= 0, else -1e30 (pre-exp add)
                    nc.gpsimd.affine_select(
                        out=maskneg[:, r, :],
                        in_=maskneg[:, r, :],
                        compare_op=mybir.AluOpType.is_ge,
                        fill=NEG,
                        base=-128 * r,
                        pattern=[[1, TCH]],
                        channel_multiplier=-1,
                    )
                # e_h: [128, 4] indicator columns; rowsum matmuls with e_h
                # stationary land head h's row-sums in row h of a shared
                # [4, TCH] PSUM bank (other rows accumulate exact zeros)
                eh_sb = w2.tile([128, NH, NH], BF16, tag="eh")
                nc.vector.memset(eh_sb, 0.0)
                for h in range(NH):
                    nc.vector.memset(eh_sb[:, h, h : h + 1], 1.0)

                def attn_chunk(j, prefetch_wo=False, after_head0=None):
                    """Attention for all 4 heads of chunk j. Row-sums of all
                    heads accumulate into one [4, TCH] PSUM bank; at the end
                    the reciprocal is taken straight from PSUM and written to
                    DRAM. The broadcast+scale finish runs later (norm_finish),
                    woven into the next chunk so the DMA latency hides."""
                    ts = slice(j * TCH, (j + 1) * TCH)
                    n_i = 4 * j + 4
                    psum_l = psl.tile([NH, TCH], F32, tag="l")
                    for h in range(NH):
                        if prefetch_wo:
                            nc.sync.dma_start(
                                out=wo_sb[:, h, :], in_=wor[:, h, :]
                            )
                        psum_ot = psot.tile([128, TCH], F32, tag="ot")
                        for i in range(n_i):
                            psum_st = pst.tile([128, TCH], F32, tag="st")
                            nc.tensor.matmul(
                                psum_st[:],
                                kt_sb[:, i * 128 : (i + 1) * 128],
                                qt_sb[h][:, ts],
                                start=True,
                                stop=True,
                            )
                            r = i - 4 * j
                            if r >= 0:  # diagonal-crossing tile
                                masked = maskedp.tile([128, TCH], F32, tag="masked")
                                nc.vector.tensor_add(
                                    masked[:], psum_st[:], maskneg[:, r, :]
                                )
                                src = masked
                            else:
                                src = psum_st
                            phat = phatp.tile([128, TCH], BF16, tag="phat")
                            nc.scalar.activation(
                                out=phat[:],
                                in_=src[:],
                                func=mybir.ActivationFunctionType.Exp,
                                scale=SCALE,
                            )
                            nc.tensor.matmul(
                                psum_l[:],
                                eh_sb[:, h, :],
                                phat[:],
                                start=(h == 0 and i == 0),
                                stop=(h == NH - 1 and i == n_i - 1),
                            )
                            nc.tensor.matmul(
                                psum_ot[:],
                                v_sb[:, i, :],
                                phat[:],
                                start=(i == 0),
                                stop=(i == n_i - 1),
                            )
                            if h == 1 and i == 1 and after_head0 is not None:
                                after_head0()
                                after_head0 = None
                        # unnormalized O^T; scaling happens in norm_finish
                        nc.vector.tensor_copy(ao_sb[h][:, ts], psum_ot[:])
                    recip_sb = recipp.tile([NH, TCH], F32, tag="recip")
                    nc.vector.reciprocal(recip_sb[:], psum_l[:])
                    rd_ = rdram.tile([NH, TCH], F32, tag="rd")
                    nc.sync.dma_start(out=rd_, in_=recip_sb)
                    return rd_

                def norm_finish(j, rd_):
                    ts = slice(j * TCH, (j + 1) * TCH)
                    for h in range(NH):
                        rb = rbcp.tile([128, TCH], F32, tag="rb")
                        nc.sync.dma_start(
                            out=rb,
                            in_=rd_[h, :].partition_broadcast(128),
                        )
                        nc.vector.tensor_mul(
                            ao_sb[h][:, ts], ao_sb[h][:, ts], rb[:]
                        )

                def outproj_chunk(j):
                    for tt in range(4 * j, 4 * j + 4):
                        tsl = slice(tt * 128, (tt + 1) * 128)
                        for fc in range(8):
                            fsl = slice(fc * 512, (fc + 1) * 512)
                            py = psy.tile([128, 512], F32, tag="y")
                            for hb in range(NH):
                                nc.tensor.matmul(
                                    py[:],
                                    ao_sb[hb][:, tsl],
                                    wo_sb[:, hb, fsl],
                                    start=(hb == 0),
                                    stop=(hb == NH - 1),
                                )
                            yt = ys.tile([128, 512], BF16, tag="yt")
                            nc.vector.tensor_copy(yt[:], py[:])
                            nc.sync.dma_start(out=y[tsl, fsl], in_=yt[:])

                # schedule: norm_finish(j) is woven into attention of chunk
                # j-1 (after its first head) and out-proj of chunk j runs
                # after attention of chunk j-1, so the normalization DMA
                # hides under compute and y DMA overlaps attention
                rd3 = attn_chunk(3, prefetch_wo=True)
                rd2 = attn_chunk(2, after_head0=lambda: norm_finish(3, rd3))
                outproj_chunk(3)
                rd1 = attn_chunk(1, after_head0=lambda: norm_finish(2, rd2))
                outproj_chunk(2)
                rd0 = attn_chunk(0, after_head0=lambda: norm_finish(1, rd1))
                outproj_chunk(1)
                norm_finish(0, rd0)
                outproj_chunk(0)

    nc.compile()
    return nc


def kernel(x, wq, wk, wv, wo):
    import ml_dtypes

    x = np.asarray(x, dtype=np.float32)
    wq = np.asarray(wq, dtype=np.float32)
    wk = np.asarray(wk, dtype=np.float32)
    wv = np.asarray(wv, dtype=np.float32)
    wo = np.asarray(wo, dtype=np.float32)

    if "nc" not in _CACHE:
        _CACHE["nc"] = _build()
    nc = _CACHE["nc"]

    bf16 = ml_dtypes.bfloat16
    xT = np.ascontiguousarray(x[0].T).astype(bf16)  # [DIM, T]
    ones = np.ones((128, 1), bf16)
    in_maps = []
    for c in range(NCORE):
        qs = slice(c * NH * HD, (c + 1) * NH * HD)
        ks = slice(c * HD, (c + 1) * HD)
        in_maps.append(
            {
                "xT": xT,
                "wqT": np.ascontiguousarray(wq[qs, :].T).astype(bf16),
                "wkT": np.ascontiguousarray(wk[ks, :].T).astype(bf16),
                "wvT": np.ascontiguousarray(wv[ks, :].T).astype(bf16),
                "woT": np.ascontiguousarray(wo[:, qs].T).astype(bf16),
                "ones": ones,
            }
        )

    res = run_bass_kernel_spmd(
        nc, in_maps, core_ids=list(range(NCORE)), trace=TRACE
    )
    LAST["results"] = res

    out = np.zeros((T, DIM), dtype=np.float64)
    for c in range(NCORE):
        out += res.results[c]["y"].astype(np.float64)
    return out.astype(np.float32).reshape(1, T, DIM)


# revision 16
# speedup vs baseline: 1.2302x; 1.2302x over previous
"""GQA prefill attention (B=1, T=2048, DIM=4096, 32 q-heads / 8 kv-heads),
tensor-parallel over 8 NeuronCores.

Sharding: core c owns kv head c and its 4 query heads: wq rows
[512c, 512c+512), wk/wv rows [128c, 128c+128), wo cols [512c, 512c+512).
Each core computes a partial y = attn_c @ wo_c.T in [T, DIM]; the host sums
the 8 partials (the "all-reduce after wo").

All matmul operands are bf16 (PSUM accumulation in f32); the PE streams
1 row/cycle for bf16, so precision is traded only for DMA bandwidth.
Everything on chip is kept transposed ([feature, t] layouts) so the PE
contraction dim (partitions) always lines up; the only transpose is V
(16 PE-transposes of 128x128).

Per-core pipeline:
  phase 1: Q^T/K^T/V^T projections from x^T (host-pretransposed bf16)
  phase 2+3 fused, per t-chunk j = 3..0:
    attention for all 4 heads of chunk j (S^T tiles -> exp -> rowsum via
    ones-matmul -> O^T accumulation), with NORMALIZATION DEFERRED: the
    per-chunk row-sums are batch-reciprocated off the critical path
    (DRAM-bounce transpose -> [128,16] reciprocal -> bounce back ->
    partition-broadcast reads -> in-place scale of ao), overlapping the
    next chunk's attention; out-projection + y write of chunk j runs
    after attention of chunk j-1, so y DMA overlaps attention compute.
"""

import sys

sys.path.insert(0, "/opt/trn_rl_repo")

import numpy as np

import concourse.bass as bass
import concourse.tile as tile
from concourse import bacc, mybir
from concourse.bass_utils import run_bass_kernel_spmd
from concourse.masks import make_identity

T = 2048
DIM = 4096
HD = 128
NCORE = 8
NH = 4  # q heads per core
TCH = 512
NTCH = T // TCH  # 4 t-chunks
NST = T // 128  # 16 s-tiles
NDT = DIM // 128  # 32 d-tiles
F32 = mybir.dt.float32
BF16 = mybir.dt.bfloat16
SCALE = 1.0 / float(np.sqrt(HD))
NEG = -1e30

# test.py can flip these before calling kernel() to get profiling info
TRACE = False
LAST = {}

_CACHE = {}


def _build():
    nc = bacc.Bacc("TRN2", target_bir_lowering=False, debug=False, num_devices=NCORE)
    xT = nc.dram_tensor("xT", [DIM, T], BF16, kind="ExternalInput").ap()
    wqT = nc.dram_tensor("wqT", [DIM, NH * HD], BF16, kind="ExternalInput").ap()
    wkT = nc.dram_tensor("wkT", [DIM, HD], BF16, kind="ExternalInput").ap()
    wvT = nc.dram_tensor("wvT", [DIM, HD], BF16, kind="ExternalInput").ap()
    woT = nc.dram_tensor("woT", [NH * HD, DIM], BF16, kind="ExternalInput").ap()
    ones_in = nc.dram_tensor("ones", [128, 1], BF16, kind="ExternalInput").ap()
    y = nc.dram_tensor("y", [T, DIM], BF16, kind="ExternalOutput").ap()

    with tile.TileContext(nc) as tc:
        with tc.tile_pool(name="persist", bufs=1) as persist:
            qt_sb = [persist.tile([128, T], BF16, tag=f"qt{h}", name=f"qt{h}") for h in range(NH)]
            kt_sb = persist.tile([128, T], BF16, tag="kt")
            vt_sb = persist.tile([128, T], BF16, tag="vt")
            ao_sb = [persist.tile([128, T], BF16, tag=f"ao{h}", name=f"ao{h}") for h in range(NH)]
            ones_sb = persist.tile([128, 1], BF16, tag="ones")
            ident = persist.tile([128, 128], BF16, tag="ident")
            v_sb = persist.tile([128, NST, HD], BF16, tag="v")
            nc.sync.dma_start(out=ones_sb, in_=ones_in)
            make_identity(nc, ident)

            # ---------------- phase 1: Q/K/V projections ----------------
            with (
                tc.tile_pool(name="w1", bufs=1) as w1,
                tc.tile_pool(name="xs", bufs=6) as xs,
                tc.tile_pool(name="psp", bufs=1, space="PSUM") as psp,
                tc.tile_pool(name="ptr1", bufs=1, space="PSUM") as ptr1,
            ):
                wq_sb = w1.tile([128, NDT, NH * HD], BF16, tag="wq")
                wk_sb = w1.tile([128, NDT, HD], BF16, tag="wk")
                wv_sb = w1.tile([128, NDT, HD], BF16, tag="wv")
                wqr = wqT.rearrange("(db p) f -> p db f", p=128)
                wkr = wkT.rearrange("(db p) f -> p db f", p=128)
                wvr = wvT.rearrange("(db p) f -> p db f", p=128)
                for ch in range(NTCH):
                    cs = slice(ch * TCH, (ch + 1) * TCH)
                    qps = [
                        psp.tile([128, TCH], F32, tag=f"projq{fq}", name=f"projq{fq}")
                        for fq in range(NH)
                    ]
                    kps = psp.tile([128, TCH], F32, tag="projk")
                    vps = psp.tile([128, TCH], F32, tag="projv")
                    for d in range(NDT):
                        if ch == 0 and (d in (0, 1) or (d >= 4 and d % 4 == 0)):
                            # first two tiny groups so the PE starts almost
                            # immediately; 1MB groups once streaming
                            g = slice(d, d + (1 if d == 0 else 3 if d == 1 else 4))
                            nc.sync.dma_start(out=wq_sb[:, g, :], in_=wqr[:, g, :])
                            nc.sync.dma_start(out=wk_sb[:, g, :], in_=wkr[:, g, :])
                            nc.sync.dma_start(out=wv_sb[:, g, :], in_=wvr[:, g, :])
                        xt = xs.tile([128, TCH], BF16, tag="xt")
                        nc.sync.dma_start(
                            out=xt,
                            in_=xT[d * 128 : (d + 1) * 128, cs],
                        )
                        st = d == 0
                        sp = d == NDT - 1
                        for fq in range(NH):
                            nc.tensor.matmul(
                                qps[fq][:],
                                wq_sb[:, d, fq * HD : (fq + 1) * HD],
                                xt[:],
                                start=st,
                                stop=sp,
                            )
                        nc.tensor.matmul(
                            kps[:], wk_sb[:, d, :], xt[:], start=st, stop=sp
                        )
                        nc.tensor.matmul(
                            vps[:], wv_sb[:, d, :], xt[:], start=st, stop=sp
                        )
                    for fq in range(NH):
                        nc.vector.tensor_copy(qt_sb[fq][:, cs], qps[fq][:])
                    nc.vector.tensor_copy(kt_sb[:, cs], kps[:])
                    nc.vector.tensor_copy(vt_sb[:, cs], vps[:])
                    for ii in range(4 * ch, 4 * ch + 4):
                        ptr = ptr1.tile([128, HD], BF16, tag="tr")
                        nc.tensor.transpose(
                            ptr[:], vt_sb[:, ii * 128 : (ii + 1) * 128], ident[:]
                        )
                        nc.vector.tensor_copy(v_sb[:, ii, :], ptr[:])

            # ---------------- phase 2+3: attention + out-projection ----------------
            with (
                tc.tile_pool(name="w2", bufs=1) as w2,
                tc.tile_pool(name="phatp", bufs=6) as phatp,
                tc.tile_pool(name="recipp", bufs=2) as recipp,
                tc.tile_pool(name="rbcp", bufs=3) as rbcp,
                tc.tile_pool(name="rdram", bufs=2, space="DRAM") as rdram,
                tc.tile_pool(name="ys", bufs=4) as ys,
                tc.tile_pool(name="maskedp", bufs=3) as maskedp,
                tc.tile_pool(name="pst", bufs=2, space="PSUM") as pst,
                tc.tile_pool(name="psl", bufs=2, space="PSUM") as psl,
                tc.tile_pool(name="psot", bufs=2, space="PSUM") as psot,
                tc.tile_pool(name="psy", bufs=2, space="PSUM") as psy,
            ):
                wo_sb = w2.tile([128, NH, DIM], BF16, tag="wo")
                wor = woT.rearrange("(hb p) f -> p hb f", p=128)
                maskneg = w2.tile([128, 4, TCH], F32, tag="maskneg")
                nc.vector.memset(maskneg, 0.0)
                for r in range(4):
                    # keep where (t - s - 128r) >= 0, else -1e30 (pre-exp add)
                    nc.gpsimd.affine_select(
                        out=maskneg[:, r, :],
                        in_=maskneg[:, r, :],
                        compare_op=mybir.AluOpType.is_ge,
                        fill=NEG,
                        base=-128 * r,
                        pattern=[[1, TCH]],
                        channel_multiplier=-1,
                    )
                # e_h: [128, 4] indicator columns; rowsum matmuls with e_h
                # stationary land head h's row-sums in row h of a shared
                # [4, TCH] PSUM bank (other rows accumulate exact zeros)
                eh_sb = w2.tile([128, NH, NH], BF16, tag="eh")
                nc.vector.memset(eh_sb, 0.0)
                for h in range(NH):
                    nc.vector.memset(eh_sb[:, h, h : h + 1], 1.0)

                def attn_chunk(j, prefetch_wo=False, after_head0=None):
                    """Attention for all 4 heads of chunk j. Row-sums of all
                    heads accumulate into one [4, TCH] PSUM bank; at the end
                    the reciprocal is taken straight from PSUM and written to
                    DRAM. The broadcast+scale finish runs later (norm_finish),
                    woven into the next chunk so the DMA latency hides."""
                    ts = slice(j * TCH, (j + 1) * TCH)
                    n_i = 4 * j + 4
                    psum_l = psl.tile([NH, TCH], F32, tag="l")
                    for h in range(NH):
                        if prefetch_wo:
                            nc.sync.dma_start(
                                out=wo_sb[:, h, :], in_=wor[:, h, :]
                            )
                        psum_ot = psot.tile([128, TCH], F32, tag="ot")
                        for i in range(n_i):
                            psum_st = pst.tile([128, TCH], F32, tag="st")
                            nc.tensor.matmul(
                                psum_st[:],
                                kt_sb[:, i * 128 : (i + 1) * 128],
                                qt_sb[h][:, ts],
                                start=True,
                                stop=True,
                            )
                            r = i - 4 * j
                            if r >= 0:  # diagonal-crossing tile
                                masked = maskedp.tile([128, TCH], F32, tag="masked")
                                nc.vector.tensor_add(
                                    masked[:], psum_st[:], maskneg[:, r, :]
                                )
                                src = masked
                            else:
                                src = psum_st
                            phat = phatp.tile([128, TCH], BF16, tag="phat")
                            nc.scalar.activation(
                                out=phat[:],
                                in_=src[:],
                                func=mybir.ActivationFunctionType.Exp,
                                scale=SCALE,
                            )
                            nc.tensor.matmul(
                                psum_l[:],
                                eh_sb[:, h, :],
                                phat[:],
                                start=(h == 0 and i == 0),
                                stop=(h == NH - 1 and i == n_i - 1),
                            )
                            nc.tensor.matmul(
                                psum_ot[:],
                                v_sb[:, i, :],
                                phat[:],
                                start=(i == 0),
                                stop=(i == n_i - 1),
                            )
                            if h == 1 and i == 1 and after_head0 is not None:
                                after_head0()
                                after_head0 = None
                        # unnormalized O^T; scaling happens in norm_finish
                        nc.vector.tensor_copy(ao_sb[h][:, ts], psum_ot[:])
                    recip_sb = recipp.tile([NH, TCH], F32, tag="recip")
                    nc.vector.reciprocal(recip_sb[:], psum_l[:])
                    rd_ = rdram.tile([NH, TCH], F32, tag="rd")
                    nc.sync.dma_start(out=rd_, in_=recip_sb)
                    return rd_

                def norm_finish(j, rd_):
                    ts = slice(j * TCH, (j + 1) * TCH)
                    for h in range(NH):
                        rb = rbcp.tile([128, TCH], F32, tag="rb")
                        nc.sync.dma_start(
                            out=rb,
                            in_=rd_[h, :].partition_broadcast(128),
                        )
                        nc.vector.tensor_mul(
                            ao_sb[h][:, ts], ao_sb[h][:, ts], rb[:]
                        )

                def outproj_chunk(j):
                    for tt in range(4 * j, 4 * j + 4):
                        tsl = slice(tt * 128, (tt + 1) * 128)
                        for fc in range(8):
                            fsl = slice(fc * 512, (fc + 1) * 512)
                            py = psy.tile([128, 512], F32, tag="y")
                            for hb in range(NH):
                                nc.tensor.matmul(
                                    py[:],
                                    ao_sb[hb][:, tsl],
                                    wo_sb[:, hb, fsl],
                                    start=(hb == 0),
                                    stop=(hb == NH - 1),
                                )
                            yt = ys.tile([128, 512], BF16, tag="yt")
                            nc.vector.tensor_copy(yt[:], py[:])
                            nc.sync.dma_start(out=y[tsl, fsl], in_=yt[:])

                # schedule: norm_finish(j) is woven into attention of chunk
                # j-1 (after its first head) and out-proj of chunk j runs
                # after attention of chunk j-1, so the normalization DMA
                # hides under compute and y DMA overlaps attention
                rd3 = attn_chunk(3, prefetch_wo=True)
                rd2 = attn_chunk(2, after_head0=lambda: norm_finish(3, rd3))
                outproj_chunk(3)
                rd1 = attn_chunk(1, after_head0=lambda: norm_finish(2, rd2))
                outproj_chunk(2)
                rd0 = attn_chunk(0, after_head0=lambda: norm_finish(1, rd1))
                outproj_chunk(1)
                norm_finish(0, rd0)
                outproj_chunk(0)

    nc.compile()
    return nc


def kernel(x, wq, wk, wv, wo):
    import ml_dtypes

    x = np.asarray(x, dtype=np.float32)
    wq = np.asarray(wq, dtype=np.float32)
    wk = np.asarray(wk, dtype=np.float32)
    wv = np.asarray(wv, dtype=np.float32)
    wo = np.asarray(wo, dtype=np.float32)

    if "nc" not in _CACHE:
        _CACHE["nc"] = _build()
    nc = _CACHE["nc"]

    bf16 = ml_dtypes.bfloat16
    xT = np.ascontiguousarray(x[0].T).astype(bf16)  # [DIM, T]
    ones = np.ones((128, 1), bf16)
    in_maps = []
    for c in range(NCORE):
        qs = slice(c * NH * HD, (c + 1) * NH * HD)
        ks = slice(c * HD, (c + 1) * HD)
        in_maps.append(
            {
                "xT": xT,
                "wqT": np.ascontiguousarray(wq[qs, :].T).astype(bf16),
                "wkT": np.ascontiguousarray(wk[ks, :].T).astype(bf16),
                "wvT": np.ascontiguousarray(wv[ks, :].T).astype(bf16),
                "woT": np.ascontiguousarray(wo[:, qs].T).astype(bf16),
                "ones": ones,
            }
        )

    res = run_bass_kernel_spmd(
        nc, in_maps, core_ids=list(range(NCORE)), trace=TRACE
    )
    LAST["results"] = res

    out = np.zeros((T, DIM), dtype=np.float64)
    for c in range(NCORE):
        out += res.results[c]["y"].astype(np.float64)
    return out.astype(np.float32).reshape(1, T, DIM)
